# revision 3
# baseline (speedup 1.0000x reference)
"""GRUDecoder kernel: data-parallel over the agent axis N across 8 NeuronCores.

Sharding strategy (per spec hint): each core holds N/8 = 128 agents — all
modes / timesteps local, parameters replicated. Scene size is 64, so each
128-agent shard holds exactly 2 complete scenes and the per-scene attention
is fully shard-local. The full network (proj -> conv decoder -> GRU enc block
-> per-scene MHSA -> GRU dec block -> loc head) runs independently per shard;
the host only splits inputs and concatenates outputs.

Execution backend: tries the neuron (axon PJRT) devices first with a bounded
compile budget (compile artifacts are cached by neuronx-cc, so a warm cache
makes this fast); falls back to CPU execution of the identical math if device
compilation does not finish in time, so the kernel always returns a correct
full-shape output.
"""
import os
import threading
import numpy as np

H = 128
M = 6
OBS = 20
PRED = 30
HEADS = 8
HD = H // HEADS
NCORES = 8

# seconds allowed for neuron compile before falling back to CPU
_NEURON_COMPILE_BUDGET = float(os.environ.get("GRUDEC_NEURON_BUDGET", "600"))
_BACKEND = os.environ.get("GRUDEC_BACKEND", "auto")  # auto | neuron | cpu

_STATE = {}


def _np_params(params):
    import jax
    return jax.tree_util.tree_map(lambda x: np.asarray(x, dtype=np.float32), params)


# ---------------- network math (mirrors the reference exactly) ----------------

def _make_forward():
    import jax
    import jax.numpy as jnp

    def _lin(x, p):
        return x @ p['w'].T + p['b']

    def _ln(x, p, eps=1e-5):
        mu = jnp.mean(x, axis=-1, keepdims=True)
        var = jnp.mean((x - mu) ** 2, axis=-1, keepdims=True)
        return (x - mu) / jnp.sqrt(var + eps) * p['g'] + p['b']

    def _gru(p, x, h):
        gi = x @ p['wih'].T + p['bih']
        gh = h @ p['whh'].T + p['bhh']
        ir, iz, inn = jnp.split(gi, 3, axis=-1)
        hr, hz, hn = jnp.split(gh, 3, axis=-1)
        r = jax.nn.sigmoid(ir + hr)
        z = jax.nn.sigmoid(iz + hz)
        n = jnp.tanh(inn + r * hn)
        return (1.0 - z) * n + z * h

    def _mlp(x, p):
        x = _ln(_lin(x, p['l1']), p['ln1'])
        x = _ln(_lin(x, p['l2']), p['ln2'])
        return jax.nn.relu(x)

    def _block(p, temporal, h_temp, h, goal, refinement):
        h_sq = h[0]

        def mid(c, inp):
            x, r = inp
            c = _gru(p['gru_mid'], x, c + r)
            return c, c

        _, v_mid = jax.lax.scan(mid, h_sq, (temporal, refinement))
        v_for = h + _mlp(v_mid, p['mlp_for'])

        def fstep(c, x):
            c = _gru(p['gru_for'], x, c)
            return c, c

        _, out_for = jax.lax.scan(fstep, h_temp, v_for)
        v_back = h + _mlp(v_mid, p['mlp_bac'])
        hb0 = jax.nn.relu(_ln(_lin(goal, p['goal_h']['l']), p['goal_h']['ln'])).reshape(-1, H)

        def bstep(c, x):
            c = _gru(p['gru_back'], x, c)
            return c, c

        _, out_back = jax.lax.scan(bstep, hb0, v_back, reverse=True)
        f = out_for.transpose(1, 0, 2)
        b = out_back.transpose(1, 0, 2)
        w = jax.nn.sigmoid(_ln(_lin(jnp.concatenate([f, b], axis=-1), p['fuse']['l']),
                               p['fuse']['ln']))
        return f * w + b * (1.0 - w)

    def _mhsa(x, p):
        B, S, _ = x.shape
        q = _lin(x, p['q']).reshape(B, S, HEADS, HD).transpose(0, 2, 1, 3)
        k = _lin(x, p['k']).reshape(B, S, HEADS, HD).transpose(0, 2, 1, 3)
        v = _lin(x, p['v']).reshape(B, S, HEADS, HD).transpose(0, 2, 1, 3)
        scores = jnp.einsum('bhqd,bhkd->bhqk', q, k) / (HD ** 0.5)
        attn = jax.nn.softmax(scores, axis=-1)
        ctx = jnp.einsum('bhqk,bhkd->bhqd', attn, v).transpose(0, 2, 1, 3).reshape(B, S, H)
        return _lin(ctx, p['o'])

    def forward(temporal_feature, interaction_feature, goal, params, scene_size):
        n = temporal_feature.shape[0]
        his = jax.nn.relu(_ln(_lin(temporal_feature, params['proj_global']['l']),
                              params['proj_global']['ln']))
        his = his.reshape(n, OBS, M, H).transpose(1, 2, 0, 3).reshape(OBS, M * n, H)
        temporal = jnp.einsum('po,onh->pnh', params['conv']['wt'], his) + params['conv']['bt']
        his_social = jnp.broadcast_to(interaction_feature[None], (M, n, H)).reshape(1, M * n, H)
        goal_p = jax.nn.relu(_ln(_lin(goal, params['proj_goal']['l']), params['proj_goal']['ln']))
        goal_p = goal_p.reshape(n, M, H).transpose(1, 0, 2)
        enc = _block(params['enc'], temporal, his[-1], his_social, goal_p,
                     jnp.zeros_like(temporal))
        temporal_1 = enc.transpose(1, 0, 2)
        tmp = temporal_1.reshape(PRED, M, n, H).transpose(2, 1, 0, 3).reshape(n, M * PRED, H)
        ns = n // scene_size
        x = tmp.reshape(ns, scene_size, M * PRED, H).transpose(0, 2, 1, 3).reshape(
            ns * M * PRED, scene_size, H)
        ref = _mhsa(x, params['attn']).reshape(ns, M * PRED, scene_size, H).transpose(
            0, 2, 1, 3).reshape(n, M * PRED, H)
        ref = ref.reshape(n, M, PRED, H).transpose(2, 1, 0, 3).reshape(PRED, M * n, H)
        dec = _block(params['dec'], temporal, his[-1], his_social, goal_p, ref)
        out = _lin(jax.nn.relu(_ln(_lin(dec, params['loc']['l1']), params['loc']['ln'])),
                   params['loc']['l2'])
        return out.reshape(M, n, PRED, 2)

    return forward


def _example_shard_args(nloc, params):
    z = np.zeros((nloc, OBS, H), np.float32)
    zi = np.zeros((nloc, H), np.float32)
    return (z, zi, zi.copy(), params)


def _try_neuron(ssz, nloc, params):
    """Compile the shard forward for the neuron backend within the time
    budget. Returns (jfn, devices) or None."""
    import jax

    try:
        devs = [d for d in jax.devices() if d.platform != 'cpu']
    except Exception:
        return None
    if len(devs) < NCORES:
        return None
    devs = devs[:NCORES]

    jfn = jax.jit(_make_forward(), static_argnames=('scene_size',))
    result = {}

    def compile_worker():
        try:
            args = jax.device_put(_example_shard_args(nloc, params), devs[0])
            lowered = jfn.lower(*args, scene_size=ssz)
            result['exe'] = lowered.compile()
        except Exception as e:  # compile failure -> fallback
            result['err'] = e

    th = threading.Thread(target=compile_worker, daemon=True)
    th.start()
    th.join(_NEURON_COMPILE_BUDGET)
    if th.is_alive() or 'exe' not in result:
        return None
    return jfn, devs


def _get_exec(ssz, nloc, params):
    key = (ssz, nloc)
    if key in _STATE:
        return _STATE[key]
    import jax

    neuron = None
    if _BACKEND in ("auto", "neuron"):
        try:
            neuron = _try_neuron(ssz, nloc, params)
        except Exception:
            neuron = None

    if neuron is not None:
        _STATE[key] = neuron
    else:
        cpu = jax.devices('cpu')[0]
        jfn = jax.jit(_make_forward(), static_argnames=('scene_size',))
        _STATE[key] = (jfn, [cpu] * NCORES)
    return _STATE[key]


def kernel(temporal_feature, interaction_feature, goal, params, scene_size):
    import jax

    temporal_feature = np.asarray(temporal_feature, dtype=np.float32)
    interaction_feature = np.asarray(interaction_feature, dtype=np.float32)
    goal = np.asarray(goal, dtype=np.float32)
    params = _np_params(params)
    ssz = int(scene_size)

    n = temporal_feature.shape[0]
    nloc = n // NCORES
    assert nloc % ssz == 0, "shard must hold whole scenes"

    jfn, devs = _get_exec(ssz, nloc, params)

    futs = []
    for c, d in enumerate(devs):
        sl = slice(c * nloc, (c + 1) * nloc)
        args = jax.device_put(
            (temporal_feature[sl], interaction_feature[sl], goal[sl], params), d)
        futs.append(jfn(*args, scene_size=ssz))
    shard_outs = [np.asarray(o) for o in futs]

    return np.concatenate(shard_outs, axis=1).astype(np.float32)
